# revision 13
# baseline (speedup 1.0000x reference)
"""DeepSeekMoE layer on 8 TRN2 NeuronCores — expert-parallel, fp16 fused pipeline.

Reference computation (per token):
    shared = silu(x @ ws1) @ ws2
    router: softmax(x @ w_router) -> top-2 -> renormalize -> gates
    routed = sum_{e in top2} gate_e * silu(x @ w1[e]) @ w2[e]
    out    = shared + routed

Sharding: expert-parallel. Core e receives the (padded to 128) bucket of all
token rows routed to expert e (capacity C), plus a 1/8 slice of all tokens for
the shared expert, packed into one row stream [routed | shared]. Routing
(softmax/top-k) and the dispatch/combine permutations run on the host; all
GEMMs + SiLU + gate scaling run on device.

Device kernel: all matmul operands are fp16 (measured ~227 ns/MM sustained at
512 moving columns vs 256 ns for f32r — FWL hides the stationary load for
16-bit dtypes). Work proceeds in fused R-tiles of 512 rows: pass1 computes
hT = silu(w1.T @ xT) into an SBUF-resident fp16 tile (no DRAM round-trip),
pass2 immediately contracts it with w2, scales by the per-token gate and
stores y as fp16. PSUM accumulates in fp32 throughout.
"""

import numpy as np
import ml_dtypes

import concourse.mybir as mybir
import concourse.tile as tile
from concourse import bacc
from concourse.bass_utils import run_bass_kernel_spmd

H = 2048          # hidden
I = 1408          # moe intermediate
E = 8             # routed experts == n cores
NCORES = 8
RT = 512          # token tile (rows) per fused step
KH = H // 128     # 16 k-blocks over hidden
KI = I // 128     # 11 k-blocks over intermediate
F32 = mybir.dt.float32
F16 = mybir.dt.float16
NPF16 = np.float16

_BUILD_CACHE: dict = {}


def _tiles(ncols):
    """R-tile (offset, size) list: one 512-tile, optional 256/128 tail, then
    the remaining 512-tiles.

    A full-size first tile gives pass1 enough runtime to cover the w2 weight
    DMA before the first pass2 needs it; a full-size last tile covers the next
    phase's weight DMAs with its pass2."""
    tail, off = [], 0
    while (ncols - off) % RT:
        sz = 128 if (ncols - off) % 256 else 256
        assert (ncols - off) % 128 == 0
        tail.append((off, sz))
        off += sz
    full = [(o, RT) for o in range(off, ncols, RT)]
    if full and tail:
        return full[:1] + tail + full[1:]
    return tail + full


def _load_xtile(nc, in_pool, xb, off, sz, ph):
    xt = in_pool.tile([128, KH, RT], F16, tag="xin", name=f"x_{ph}")
    nc.sync.dma_start(xt[:, :, :sz], xb[:, :, off:off + sz])
    return xt


def _emit_rtile(nc, pools, w1t, w2t, xt, yb, scale_sb, off, sz, ph, inv_s):
    """One fused R-tile: pass1 (h in SBUF) then pass2 (gate-scaled store).

    xt: SBUF [128, KH, RT] (pre-loaded); yb: DRAM [128, Ctot//128, H].
    scale_sb: per-token gates [128, Ctot//128] in SBUF, or None (shared rows,
    plain copy scaled by inv_s).
    """
    in_pool, h_pool, out_pool, psum_pool = pools

    ht = h_pool.tile([128, KI, RT], F16, tag="h", name=f"h_{ph}")

    # pass1: hT[i,:] = silu((x @ w1)[i,:]) with I on partitions, tokens free
    for i in range(KI):
        ps = psum_pool.tile([128, sz], F32, tag="ps1", name=f"ps1_{ph}_{i}")
        for k in range(KH):
            nc.tensor.matmul(ps[:], w1t[:, k, i * 128:(i + 1) * 128],
                             xt[:, k, :sz], start=(k == 0), stop=(k == KH - 1))
        nc.scalar.activation(ht[:, i, :sz], ps[:],
                             mybir.ActivationFunctionType.Silu, scale=inv_s)

    # pass2: y[m,:] = (h.T @ w2) * gate[m], tokens on partitions
    for c in range(sz // 128):
        m = off // 128 + c
        yt = out_pool.tile([128, H], F16, tag="yt", name=f"y_{ph}_{c}")
        for hb in range(H // 512):
            ps = psum_pool.tile([128, 512], F32, tag="ps2", name=f"ps2_{ph}_{c}_{hb}")
            for i in range(KI):
                nc.tensor.matmul(ps[:], ht[:, i, c * 128:(c + 1) * 128],
                                 w2t[:, i, hb * 512:(hb + 1) * 512],
                                 start=(i == 0), stop=(i == KI - 1))
            if scale_sb is not None:
                nc.vector.tensor_scalar_mul(yt[:, hb * 512:(hb + 1) * 512], ps[:],
                                            scale_sb[:, m:m + 1])
            else:
                nc.vector.tensor_scalar_mul(yt[:, hb * 512:(hb + 1) * 512], ps[:],
                                            inv_s)
        nc.gpsimd.dma_start(yb[:, m, :], yt[:])


def build(C, S, debug=False, reps=1):
    """Per-core Bass module. C: routed capacity, S: shared rows (both %128==0).

    reps>1 repeats the whole computation in one NEFF (timing use only)."""
    assert C % 128 == 0 and S % 128 == 0
    Ctot = C + S
    nc = bacc.Bacc(None, target_bir_lowering=False, debug=debug)
    with tile.TileContext(nc) as tc:
        with tc.tile_pool(name="dram", bufs=1, space="DRAM") as dram:
            xb = dram.tile((128, KH, Ctot), F16, kind="ExternalInput", name="xb", uniquify=False)
            w1e = dram.tile((128, KH, I), F16, kind="ExternalInput", name="w1e", uniquify=False)
            w2e = dram.tile((128, KI, H), F16, kind="ExternalInput", name="w2e", uniquify=False)
            ws1 = dram.tile((128, KH, I), F16, kind="ExternalInput", name="ws1", uniquify=False)
            ws2 = dram.tile((128, KI, H), F16, kind="ExternalInput", name="ws2", uniquify=False)
            gate = dram.tile((128, C // 128), F32, kind="ExternalInput", name="gate", uniquify=False)
            yb = dram.tile((128, Ctot // 128, H), F16, kind="ExternalOutput", name="yb", uniquify=False)

            with (
                tc.tile_pool(name="w1pool", bufs=1) as w1pool,
                tc.tile_pool(name="w2pool", bufs=1) as w2pool,
                tc.tile_pool(name="inpool", bufs=4) as in_pool,
                tc.tile_pool(name="hpool", bufs=2) as h_pool,
                tc.tile_pool(name="outpool", bufs=4) as out_pool,
                tc.tile_pool(name="psum", bufs=4, space="PSUM") as psum_pool,
                tc.tile_pool(name="const", bufs=1) as const_pool,
            ):
                pools = (in_pool, h_pool, out_pool, psum_pool)
                scale_sb = const_pool.tile([128, C // 128], F32, name="scale_sb")
                nc.sync.dma_start(scale_sb[:], gate[:])

                for rep in range(reps):
                    for phase, (w1d, w2d, lo, n, sc) in enumerate((
                            (w1e, w2e, 0, C, scale_sb),
                            (ws1, ws2, C, S, None))):
                        tiles = _tiles(n)
                        # phase's first x tiles load BEFORE the weight DMAs so
                        # they aren't stuck behind them in the sync ring
                        xts = [_load_xtile(nc, in_pool, xb, lo + off, sz,
                                           f"{rep}_{phase}_{r}")
                               for r, (off, sz) in enumerate(tiles[:2])]
                        w1t = w1pool.tile([128, KH, I], F16, tag="w1",
                                          name=f"w1_{rep}_{phase}")
                        w2t = w2pool.tile([128, KI, H], F16, tag="w2",
                                          name=f"w2_{rep}_{phase}")
                        for k in range(KH):
                            nc.sync.dma_start(w1t[:, k, :], w1d[:, k, :])
                        for i in range(KI):
                            nc.sync.dma_start(w2t[:, i, :], w2d[:, i, :])
                        for r, (off, sz) in enumerate(tiles):
                            if r + 2 < len(tiles):
                                o2, s2 = tiles[r + 2]
                                xts.append(_load_xtile(
                                    nc, in_pool, xb, lo + o2, s2,
                                    f"{rep}_{phase}_{r + 2}"))
                            _emit_rtile(nc, pools, w1t, w2t, xts[r], yb, sc,
                                        lo + off, sz, f"{rep}_{phase}_{r}",
                                        1.0)
    nc.compile()
    return nc


def _get_built(C, S):
    key = (C, S)
    if key not in _BUILD_CACHE:
        _BUILD_CACHE[key] = build(C, S)
    return _BUILD_CACHE[key]


def _to_kxm_layout(a, dtype=NPF16):
    """[K, M] -> [128, K/128, M] with logical row k at (k%128, k//128)."""
    k, m_ = a.shape
    return np.ascontiguousarray(
        a.reshape(k // 128, 128, m_).transpose(1, 0, 2).astype(dtype))


def route_and_dispatch(xf, w_router):
    """Host router: returns (sorted token ids, gates, per-expert offsets, capacity)."""
    T = xf.shape[0]
    logits = xf @ w_router                       # [T, E]
    order = np.argsort(-logits, axis=1, kind="stable")[:, :2]
    mx = logits.max(axis=1, keepdims=True)
    p = np.exp(logits - mx)
    p /= p.sum(axis=1, keepdims=True)
    tk = np.take_along_axis(p, order, axis=1)    # [T, 2]
    g = tk / tk.sum(axis=1, keepdims=True)

    pe = order.ravel()                           # expert id per (token, slot) pair
    ptok = np.repeat(np.arange(T, dtype=np.int64), 2)
    pg = g.astype(np.float32).ravel()
    perm = np.argsort(pe, kind="stable")
    stok, sg = ptok[perm], pg[perm]
    counts = np.bincount(pe, minlength=E)
    offs = np.zeros(E + 1, dtype=np.int64)
    np.cumsum(counts, out=offs[1:])
    C = max(512, int(-(-counts.max() // 128) * 128))
    return stok, sg, offs, C


def prepare(x, w_shared1, w_shared2, w1, w2, w_router):
    """Host-side routing + dispatch. Returns (in_maps, meta)."""
    x = np.asarray(x, dtype=np.float32)
    w_router = np.asarray(w_router, dtype=np.float32)

    B, Sq, _ = x.shape
    T = B * Sq
    S = T // NCORES                              # shared-expert rows per core
    xf = x.reshape(T, H)

    stok, sg, offs, C = route_and_dispatch(xf, w_router)

    ws1_l = _to_kxm_layout(np.asarray(w_shared1, np.float32))
    ws2_l = _to_kxm_layout(np.asarray(w_shared2, np.float32))
    w1 = np.asarray(w1, np.float32)
    w2 = np.asarray(w2, np.float32)
    xh = xf.astype(NPF16)

    in_maps = []
    for e in range(NCORES):
        toks = stok[offs[e]:offs[e + 1]]
        n = len(toks)
        xd = np.zeros((C + S, H), NPF16)
        xd[:n] = xh[toks]
        xd[C:] = xh[e * S:(e + 1) * S]
        gate_v = np.zeros(C, np.float32)
        gate_v[:n] = sg[offs[e]:offs[e + 1]]
        in_maps.append({
            "xb": np.ascontiguousarray(
                xd.reshape(C + S, KH, 128).transpose(2, 1, 0)),
            "w1e": _to_kxm_layout(w1[e]),
            "w2e": _to_kxm_layout(w2[e]),
            "ws1": ws1_l,
            "ws2": ws2_l,
            "gate": np.ascontiguousarray(gate_v.reshape(C // 128, 128).T),
        })

    meta = (B, Sq, T, S, C, stok, offs)
    return in_maps, meta


def combine(results, meta):
    """Host-side gather/unshard of per-core outputs to the full output."""
    B, Sq, T, S, C, stok, offs = meta
    out = np.zeros((T, H), np.float32)
    for e in range(NCORES):
        toks = stok[offs[e]:offs[e + 1]]
        yp = results[e]["yb"].transpose(1, 0, 2).reshape(C + S, H).astype(np.float32)
        out[toks] += yp[:len(toks)]
        out[e * S:(e + 1) * S] += yp[C:]
    return out.reshape(B, Sq, H)


def kernel(x, w_shared1, w_shared2, w1, w2, w_router):
    in_maps, meta = prepare(x, w_shared1, w_shared2, w1, w2, w_router)
    C, S = meta[4], meta[3]
    nc = _get_built(C, S)
    res = run_bass_kernel_spmd(nc, in_maps, core_ids=list(range(NCORES)))
    return combine(res.results, meta)
